# revision 18
# baseline (speedup 1.0000x reference)
"""Multi-head attention (B=4, S=2048, D=1024, H=16) + output projection on 8 trn2 cores.

Sharding: no collectives. Core c handles batch c//2, query rows (c%2)*1024..+1024,
all 16 heads. Each core needs full K/V for its batch; W_out/b_out replicated.
The per-core output block [1024, 1024] is the final projected output for those
query rows, so the host just concatenates.

Per-core algorithm (all matmuls bf16, fp32 PSUM accumulation):
  - q, k, W cast fp32->bf16 via SWDGE DMA (per-128-column chunks so the first
    head pair is ready early) into DRAM scratch, then HWDGE DMA-transpose
    loads: qT/kT/WT tiles with head_dim (d) on partitions.
  - per head-pair hp (2 heads stacked on 128 partitions):
      per j-chunk jc (16 x 128 keys), per head h2:
        scoresT[j, i] matmul into psum [128, 1024] (double-buffered pool, so
        PE runs a jc ahead of ScalarE), 2 heads row-packed via tile_position
        ScalarE Exp over the 2-bank psum (scale=1/8 folded in) -> SBUF bf16
        AV matmuls: lhsT = v_aug [128 j, 65] (ones column -> softmax sums for
        free), accumulate over jc into psum [65, 1024] per head
      one DVE copy psum->SBUF releases the AV accumulator; the normalization
      (fast reciprocal of the sums row, gpsimd partition_broadcast, DVE mult)
      trails off the critical path into attT [128 d, 1024 i] bf16
  - projection: final[i, e] = attT.T @ WT accumulated over the 8 d-chunks,
    bias added on DVE from a partition-broadcast bias tile, fp32 out.
"""

import numpy as np

import concourse.bass as bass
import concourse.tile as tile
from concourse import bacc, mybir
from concourse.bass_utils import run_bass_kernel_spmd

B = 4
S = 2048
DM = 1024
H = 16
DK = 64
SCALE = DK**-0.5
I = 1024  # local query rows per core
NJC = S // 128  # 16 j-chunks
NHP = H // 2  # 8 head pairs == 8 d-chunks of the model dim

F32 = mybir.dt.float32
BF16 = mybir.dt.bfloat16


def build(nc: bass.Bass):
    q = nc.dram_tensor("q", [I, DM], F32, kind="ExternalInput").ap()
    k = nc.dram_tensor("k", [S, DM], F32, kind="ExternalInput").ap()
    v = nc.dram_tensor("v", [S, DM], F32, kind="ExternalInput").ap()
    w = nc.dram_tensor("w", [DM, DM], F32, kind="ExternalInput").ap()
    b = nc.dram_tensor("b", [DM], F32, kind="ExternalInput").ap()
    out = nc.dram_tensor("out", [I, DM], F32, kind="ExternalOutput").ap()

    # bf16 DRAM staging in column halves: few and LARGE DMAs (Tile's DMA
    # lanes serialize with multi-us completion latency per hop, so
    # instruction count dominates the prelude), while the half split still
    # lets the first head pairs' transposes start before all casts finish.
    # column splits: 128 (head pair 0, critical path), 384 (pairs 1-3), 512
    CSPLIT = [(0, 128), (128, 512), (512, 1024)]
    q_bf = [
        nc.dram_tensor(f"q_bf{i}", [I, c1 - c0], BF16).ap()
        for i, (c0, c1) in enumerate(CSPLIT)
    ]
    k_bf = [
        nc.dram_tensor(f"k_bf{i}", [S, c1 - c0], BF16).ap()
        for i, (c0, c1) in enumerate(CSPLIT)
    ]
    w_bf = nc.dram_tensor("w_bf", [DM, DM], BF16).ap()

    with tile.TileContext(nc) as tc:
        with (
            tc.tile_pool(name="persist", bufs=1) as pers,
            tc.tile_pool(name="expp", bufs=6) as expp,
            tc.tile_pool(name="avsbp", bufs=4) as avsbp,
            tc.tile_pool(name="nrmp", bufs=2) as nrmp,
            tc.tile_pool(name="finp", bufs=2) as finp,
        ):
            # ---- PE warmup: dummy matmuls so HAM un-throttles during the
            # DMA prelude (zeroed input; results never read) ----
            warm_sb = pers.tile([128, 512], BF16, name="warm_sb", tag="warm_sb")
            nc.vector.memset(warm_sb[:, :], 0.0)

            # ---- prelude: half-tensor casts + block-stacked transposed loads ----
            # vA_all: one big tile, all 16 heads' v_aug side by side, filled by
            # 2 DMAs (heads 0-3 early, 4-15 bulk)
            vA_all = pers.tile(
                [128, H * NJC * 65], BF16, name="vA_all", tag="vA_all"
            )
            vA4 = vA_all[:, :].rearrange("p (h jc e) -> p h jc e", h=H, e=65)

            # all ones columns in one strided memset
            nc.vector.memset(vA4[:, :, :, DK], 1.0)

            def load_v_h(h):
                # one head, all j-chunks (only ~1MB: just what one head pair
                # iteration of the attention loop actually consumes)
                src = v[:, h * DK : (h + 1) * DK].rearrange("(jc p) d -> p jc d", p=128)
                nc.gpsimd.dma_start(out=vA4[:, h, :, 0:DK], in_=src)

            def vA(h):
                return vA_all[:, h * NJC * 65 : (h + 1) * NJC * 65]

            # SWDGE queue order: everything just ahead of its first consumer;
            # head-pair 0's chain (cq0, ck0, v0, v1) is small and first
            nc.gpsimd.dma_start(out=q_bf[0][:, :], in_=q[:, 0:128])
            nc.gpsimd.dma_start(out=k_bf[0][:, :], in_=k[:, 0:128])
            load_v_h(0)
            load_v_h(1)
            nc.gpsimd.dma_start(out=q_bf[1][:, :], in_=q[:, 128:512])
            nc.gpsimd.dma_start(out=k_bf[1][:, :], in_=k[:, 128:512])
            for h in range(2, 6):
                load_v_h(h)
            nc.gpsimd.dma_start(out=q_bf[2][:, :], in_=q[:, 512:1024])
            nc.gpsimd.dma_start(out=k_bf[2][:, :], in_=k[:, 512:1024])
            for h in range(6, H):
                load_v_h(h)
            nc.gpsimd.dma_start(out=w_bf[:, :], in_=w[:, :])

            # HWDGE(SP) queue: block-stacked transposes (out[p, e, r] =
            # in[r, e*128+p]), one per cast chunk
            qTh, kTh = [], []
            for i, (c0, c1) in enumerate(CSPLIT):
                nb = (c1 - c0) // 128
                qt = pers.tile([128, nb, I], BF16, name=f"qTh{i}", tag=f"qTh{i}")
                nc.sync.dma_start(out=qt[:, :, :], in_=q_bf[i][:, :], transpose=True)
                qTh.append(qt)
                kt = pers.tile([128, nb, S], BF16, name=f"kTh{i}", tag=f"kTh{i}")
                nc.sync.dma_start(out=kt[:, :, :], in_=k_bf[i][:, :], transpose=True)
                kTh.append(kt)
            wT_all = pers.tile([128, NHP, DM], BF16, name="wT_all", tag="wT_all")
            nc.sync.dma_start(out=wT_all[:, :, :], in_=w_bf[:, :], transpose=True)
            bias_sb = pers.tile([1, DM], F32, name="bias_sb", tag="bias_sb")
            nc.sync.dma_start(out=bias_sb[:, :], in_=b[None, :])
            bias_bc = pers.tile([128, DM], F32, name="bias_bc", tag="bias_bc")

            def _chunk(hp):
                return (0, 0) if hp == 0 else ((1, hp - 1) if hp < 4 else (2, hp - 4))

            def qT(hp):
                i, e = _chunk(hp)
                return qTh[i][:, e, :]

            def kT(hp):
                i, e = _chunk(hp)
                return kTh[i][:, e, :]

            def wT(dc):
                return wT_all[:, dc, :]

            attT = []
            for hp in range(NHP):
                at = pers.tile([128, I], BF16, name=f"attT{hp}", tag=f"attT{hp}")
                attT.append(at)

            # warmup matmuls (no data deps beyond the memset)
            with tc.tile_pool(name="warmp", bufs=1, space="PSUM") as warmp:
                wps = warmp.tile([128, 512], F32, name="wps", tag="wps")
                for _ in range(14):
                    nc.tensor.matmul(
                        wps[:, :],
                        warm_sb[:, 0:128],
                        warm_sb[:, :],
                        start=True,
                        stop=True,
                        skip_group_check=True,
                    )

            # ---- attention (ScalarE-bound pipeline) ----
            with (
                tc.tile_pool(name="smmp", bufs=2, space="PSUM") as smmp,
                tc.tile_pool(name="avp", bufs=2, space="PSUM") as avp,
            ):
                for hp in range(NHP):
                    av = [
                        avp.tile([65, I], F32, name=f"av{hp}_{h2}", tag="av")
                        for h2 in range(2)
                    ]
                    for jc in range(NJC):
                        for h2 in range(2):
                            smm = smmp.tile(
                                [128, I], F32, name=f"smm{hp}_{jc}_{h2}", tag="smm"
                            )
                            lhsT = kT(hp)[
                                h2 * DK : (h2 + 1) * DK, jc * 128 : (jc + 1) * 128
                            ]
                            for ih in range(2):
                                nc.tensor.matmul(
                                    smm[:, ih * 512 : (ih + 1) * 512],
                                    lhsT,
                                    qT(hp)[
                                        h2 * DK : (h2 + 1) * DK,
                                        ih * 512 : (ih + 1) * 512,
                                    ],
                                    start=True,
                                    stop=True,
                                    tile_position=(h2 * DK, 0),
                                )
                            expt = expp.tile(
                                [128, I], BF16, name=f"ex{hp}_{jc}_{h2}", tag="expt"
                            )
                            nc.scalar.activation(
                                expt[:, :],
                                smm[:, :],
                                mybir.ActivationFunctionType.Exp,
                                scale=SCALE,
                            )
                            h = 2 * hp + h2
                            for ih in range(2):
                                nc.tensor.matmul(
                                    av[h2][:, ih * 512 : (ih + 1) * 512],
                                    vA(h)[:, jc * 65 : jc * 65 + 65],
                                    expt[:, ih * 512 : (ih + 1) * 512],
                                    start=(jc == 0),
                                    stop=(jc == NJC - 1),
                                    skip_group_check=True,
                                )
                    # release psum fast: copies for BOTH heads first (frees the
                    # av slots), then reciprocals, then the normalize multiplies
                    # (which may lag on the gpsimd broadcasts without blocking
                    # the next head pair)
                    avsb, rc = [], []
                    for h2 in range(2):
                        asb = avsbp.tile([DK, I], F32, name=f"avsb{hp}_{h2}", tag="avsb")
                        nc.vector.tensor_copy(asb[:, :], av[h2][0:DK, :])
                        sums = nrmp.tile([1, I], F32, name=f"sm{hp}_{h2}", tag="sums")
                        nc.vector.tensor_copy(sums[:, :], av[h2][DK : DK + 1, :])
                        r = nrmp.tile([1, I], F32, name=f"rc{hp}_{h2}", tag="rc")
                        nc.vector.reciprocal_approx_fast(r[:, :], sums[:, :])
                        avsb.append(asb)
                        rc.append(r)
                    for h2 in range(2):
                        rb = nrmp.tile([DK, I], F32, name=f"rb{hp}_{h2}", tag="rb")
                        nc.gpsimd.partition_broadcast(rb[:, :], rc[h2][0:1, :])
                        nc.vector.tensor_mul(
                            attT[hp][h2 * DK : (h2 + 1) * DK, :],
                            avsb[h2][:, :],
                            rb[:, :],
                        )

                # keep PE warm across the normalize -> projection handoff
                wps2 = smmp.tile([128, I], F32, name="wps2", tag="smm")
                for _ in range(10):
                    nc.tensor.matmul(
                        wps2[:, 0:512],
                        warm_sb[:, 0:128],
                        warm_sb[:, :],
                        start=True,
                        stop=True,
                        skip_group_check=True,
                    )

            # bias broadcast emitted late so it doesn't block the hp-loop
            # normalize broadcasts in the GpSimd FIFO
            nc.gpsimd.partition_broadcast(bias_bc[:, :], bias_sb[0:1, :])

            # ---- output projection ----
            with tc.tile_pool(name="projp", bufs=4, space="PSUM") as projp:
                for ic in range(I // 128):
                    # interleave the two ec psum banks so consecutive
                    # accumulating matmuls alternate banks and pipeline
                    pp = [
                        projp.tile([128, 512], F32, name=f"pp{ic}_{ec}", tag="pp")
                        for ec in range(2)
                    ]
                    for dc in range(NHP):
                        for ec in range(2):
                            nc.tensor.matmul(
                                pp[ec][:, :],
                                attT[dc][:, ic * 128 : (ic + 1) * 128],
                                wT(dc)[:, ec * 512 : (ec + 1) * 512],
                                start=(dc == 0),
                                stop=(dc == NHP - 1),
                                skip_group_check=True,
                            )
                    for ec in range(2):
                        fin = finp.tile([128, 512], F32, name=f"fin{ic}_{ec}", tag="fin")
                        nc.vector.tensor_add(
                            fin[:, :], pp[ec][:, :], bias_bc[:, ec * 512 : (ec + 1) * 512]
                        )
                        nc.sync.dma_start(
                            out=out[
                                ic * 128 : (ic + 1) * 128, ec * 512 : (ec + 1) * 512
                            ],
                            in_=fin[:, :],
                        )
    return nc


_NC_CACHE = {}


def _get_nc():
    if "nc" not in _NC_CACHE:
        nc = bacc.Bacc("TRN2", target_bir_lowering=False, debug=False)
        build(nc)
        nc.compile()
        _NC_CACHE["nc"] = nc
    return _NC_CACHE["nc"]


def kernel(q, k, v, W_out, b_out, _trace=False, _trace_kwargs=None):
    q = np.asarray(q, dtype=np.float32)
    k = np.asarray(k, dtype=np.float32)
    v = np.asarray(v, dtype=np.float32)
    W_out = np.ascontiguousarray(np.asarray(W_out, dtype=np.float32))
    b_out = np.ascontiguousarray(np.asarray(b_out, dtype=np.float32))

    nc = _get_nc()
    in_maps = []
    for c in range(8):
        bi, half = c // 2, c % 2
        in_maps.append(
            {
                "q": np.ascontiguousarray(q[bi, half * I : (half + 1) * I, :]),
                "k": np.ascontiguousarray(k[bi]),
                "v": np.ascontiguousarray(v[bi]),
                "w": W_out,
                "b": b_out,
            }
        )
    res = run_bass_kernel_spmd(
        nc,
        in_maps,
        core_ids=list(range(8)),
        trace=_trace,
        **(_trace_kwargs or {}),
    )
    out = np.empty((B, S, DM), np.float32)
    for c in range(8):
        bi, half = c // 2, c % 2
        out[bi, half * I : (half + 1) * I, :] = res.results[c]["out"]
    if _trace:
        return out, res
    return out


# revision 20
# speedup vs baseline: 1.1298x; 1.1298x over previous
"""Multi-head attention (B=4, S=2048, D=1024, H=16) + output projection on 8 trn2 cores.

Sharding: no collectives. Core c handles batch c//2, query rows (c%2)*1024..+1024,
all 16 heads. Each core needs full K/V for its batch; W_out/b_out replicated.
The per-core output block [1024, 1024] is the final projected output for those
query rows, so the host just concatenates.

Per-core algorithm (all matmuls bf16, fp32 PSUM accumulation):
  - q, k, W cast fp32->bf16 via SWDGE DMA (per-128-column chunks so the first
    head pair is ready early) into DRAM scratch, then HWDGE DMA-transpose
    loads: qT/kT/WT tiles with head_dim (d) on partitions.
  - per head-pair hp (2 heads stacked on 128 partitions):
      per j-chunk jc (16 x 128 keys), per head h2:
        scoresT[j, i] matmul into psum [128, 1024] (double-buffered pool, so
        PE runs a jc ahead of ScalarE), 2 heads row-packed via tile_position
        ScalarE Exp over the 2-bank psum (scale=1/8 folded in) -> SBUF bf16
        AV matmuls: lhsT = v_aug [128 j, 65] (ones column -> softmax sums for
        free), accumulate over jc into psum [65, 1024] per head
      one DVE copy psum->SBUF releases the AV accumulator; the normalization
      (fast reciprocal of the sums row, gpsimd partition_broadcast, DVE mult)
      trails off the critical path into attT [128 d, 1024 i] bf16
  - projection: final[i, e] = attT.T @ WT accumulated over the 8 d-chunks,
    bias added on DVE from a partition-broadcast bias tile, fp32 out.
"""

import numpy as np

import concourse.bass as bass
import concourse.tile as tile
from concourse import bacc, mybir
from concourse.bass_utils import run_bass_kernel_spmd

B = 4
S = 2048
DM = 1024
H = 16
DK = 64
SCALE = DK**-0.5
I = 1024  # local query rows per core
NJC = S // 128  # 16 j-chunks
NHP = H // 2  # 8 head pairs == 8 d-chunks of the model dim

F32 = mybir.dt.float32
BF16 = mybir.dt.bfloat16


def build(nc: bass.Bass):
    q = nc.dram_tensor("q", [I, DM], F32, kind="ExternalInput").ap()
    k = nc.dram_tensor("k", [S, DM], F32, kind="ExternalInput").ap()
    v = nc.dram_tensor("v", [S, DM], F32, kind="ExternalInput").ap()
    w = nc.dram_tensor("w", [DM, DM], F32, kind="ExternalInput").ap()
    b = nc.dram_tensor("b", [DM], F32, kind="ExternalInput").ap()
    out = nc.dram_tensor("out", [I, DM], F32, kind="ExternalOutput").ap()

    # bf16 DRAM staging in column halves: few and LARGE DMAs (Tile's DMA
    # lanes serialize with multi-us completion latency per hop, so
    # instruction count dominates the prelude), while the half split still
    # lets the first head pairs' transposes start before all casts finish.
    # column splits: 128 (head pair 0, critical path), 384 (pairs 1-3), 512
    CSPLIT = [(0, 128), (128, 512), (512, 1024)]
    q_bf = [
        nc.dram_tensor(f"q_bf{i}", [I, c1 - c0], BF16).ap()
        for i, (c0, c1) in enumerate(CSPLIT)
    ]
    k_bf = [
        nc.dram_tensor(f"k_bf{i}", [S, c1 - c0], BF16).ap()
        for i, (c0, c1) in enumerate(CSPLIT)
    ]
    w_bf = nc.dram_tensor("w_bf", [DM, DM], BF16).ap()

    with tile.TileContext(nc) as tc:
        with (
            tc.tile_pool(name="persist", bufs=1) as pers,
            tc.tile_pool(name="expp", bufs=6) as expp,
            tc.tile_pool(name="avsbp", bufs=4) as avsbp,
            tc.tile_pool(name="nrmp", bufs=4) as nrmp,
            tc.tile_pool(name="finp", bufs=2) as finp,
        ):
            # ---- PE warmup: dummy matmuls so HAM un-throttles during the
            # DMA prelude (zeroed input; results never read) ----
            warm_sb = pers.tile([128, 512], BF16, name="warm_sb", tag="warm_sb")
            nc.vector.memset(warm_sb[:, :], 0.0)

            # ---- prelude: half-tensor casts + block-stacked transposed loads ----
            # vA_all: one big tile, all 16 heads' v_aug side by side, filled by
            # 2 DMAs (heads 0-3 early, 4-15 bulk)
            vA_all = pers.tile(
                [128, H * NJC * 65], BF16, name="vA_all", tag="vA_all"
            )
            vA4 = vA_all[:, :].rearrange("p (h jc e) -> p h jc e", h=H, e=65)

            # all ones columns in one strided memset
            nc.vector.memset(vA4[:, :, :, DK], 1.0)

            def load_v_h(h):
                # one head, all j-chunks (only ~1MB: just what one head pair
                # iteration of the attention loop actually consumes)
                src = v[:, h * DK : (h + 1) * DK].rearrange("(jc p) d -> p jc d", p=128)
                nc.gpsimd.dma_start(out=vA4[:, h, :, 0:DK], in_=src)

            def vA(h):
                return vA_all[:, h * NJC * 65 : (h + 1) * NJC * 65]

            # SWDGE queue order: everything just ahead of its first consumer;
            # head-pair 0's chain (cq0, ck0, v0, v1) is small and first
            nc.gpsimd.dma_start(out=q_bf[0][:, :], in_=q[:, 0:128])
            nc.gpsimd.dma_start(out=k_bf[0][:, :], in_=k[:, 0:128])
            load_v_h(0)
            load_v_h(1)
            nc.gpsimd.dma_start(out=q_bf[1][:, :], in_=q[:, 128:512])
            nc.gpsimd.dma_start(out=k_bf[1][:, :], in_=k[:, 128:512])
            for h in range(2, 6):
                load_v_h(h)
            nc.gpsimd.dma_start(out=q_bf[2][:, :], in_=q[:, 512:1024])
            nc.gpsimd.dma_start(out=k_bf[2][:, :], in_=k[:, 512:1024])
            for h in range(6, H):
                load_v_h(h)
            nc.gpsimd.dma_start(out=w_bf[:, :], in_=w[:, :])

            # HWDGE(SP) queue: block-stacked transposes (out[p, e, r] =
            # in[r, e*128+p]), one per cast chunk
            qTh, kTh = [], []
            for i, (c0, c1) in enumerate(CSPLIT):
                nb = (c1 - c0) // 128
                qt = pers.tile([128, nb, I], BF16, name=f"qTh{i}", tag=f"qTh{i}")
                nc.sync.dma_start(out=qt[:, :, :], in_=q_bf[i][:, :], transpose=True)
                qTh.append(qt)
                kt = pers.tile([128, nb, S], BF16, name=f"kTh{i}", tag=f"kTh{i}")
                nc.sync.dma_start(out=kt[:, :, :], in_=k_bf[i][:, :], transpose=True)
                kTh.append(kt)
            wT_all = pers.tile([128, NHP, DM], BF16, name="wT_all", tag="wT_all")
            nc.sync.dma_start(out=wT_all[:, :, :], in_=w_bf[:, :], transpose=True)
            bias_sb = pers.tile([1, DM], F32, name="bias_sb", tag="bias_sb")
            nc.sync.dma_start(out=bias_sb[:, :], in_=b[None, :])
            bias_bc = pers.tile([128, DM], F32, name="bias_bc", tag="bias_bc")

            def _chunk(hp):
                return (0, 0) if hp == 0 else ((1, hp - 1) if hp < 4 else (2, hp - 4))

            def qT(hp):
                i, e = _chunk(hp)
                return qTh[i][:, e, :]

            def kT(hp):
                i, e = _chunk(hp)
                return kTh[i][:, e, :]

            def wT(dc):
                return wT_all[:, dc, :]

            attT = []
            for hp in range(NHP):
                at = pers.tile([128, I], BF16, name=f"attT{hp}", tag=f"attT{hp}")
                attT.append(at)

            # warmup matmuls (no data deps beyond the memset)
            with tc.tile_pool(name="warmp", bufs=1, space="PSUM") as warmp:
                wps = warmp.tile([128, 512], F32, name="wps", tag="wps")
                for _ in range(14):
                    nc.tensor.matmul(
                        wps[:, :],
                        warm_sb[:, 0:128],
                        warm_sb[:, :],
                        start=True,
                        stop=True,
                        skip_group_check=True,
                    )

            # ---- attention (ScalarE-bound pipeline) ----
            with (
                tc.tile_pool(name="smmp", bufs=2, space="PSUM") as smmp,
                tc.tile_pool(name="avp", bufs=2, space="PSUM") as avp,
            ):
                pending_mults = []
                for hp in range(NHP):
                    av = [
                        avp.tile([65, I], F32, name=f"av{hp}_{h2}", tag="av")
                        for h2 in range(2)
                    ]
                    for jc in range(NJC):
                        for h2 in range(2):
                            smm = smmp.tile(
                                [128, I], F32, name=f"smm{hp}_{jc}_{h2}", tag="smm"
                            )
                            lhsT = kT(hp)[
                                h2 * DK : (h2 + 1) * DK, jc * 128 : (jc + 1) * 128
                            ]
                            for ih in range(2):
                                nc.tensor.matmul(
                                    smm[:, ih * 512 : (ih + 1) * 512],
                                    lhsT,
                                    qT(hp)[
                                        h2 * DK : (h2 + 1) * DK,
                                        ih * 512 : (ih + 1) * 512,
                                    ],
                                    start=True,
                                    stop=True,
                                    tile_position=(h2 * DK, 0),
                                )
                            expt = expp.tile(
                                [128, I], BF16, name=f"ex{hp}_{jc}_{h2}", tag="expt"
                            )
                            nc.scalar.activation(
                                expt[:, :],
                                smm[:, :],
                                mybir.ActivationFunctionType.Exp,
                                scale=SCALE,
                            )
                            h = 2 * hp + h2
                            for ih in range(2):
                                nc.tensor.matmul(
                                    av[h2][:, ih * 512 : (ih + 1) * 512],
                                    vA(h)[:, jc * 65 : jc * 65 + 65],
                                    expt[:, ih * 512 : (ih + 1) * 512],
                                    start=(jc == 0),
                                    stop=(jc == NJC - 1),
                                    skip_group_check=True,
                                )
                    # release psum fast: copies free the av slots, reciprocals
                    # and broadcasts follow; the normalize MULTIPLIES are
                    # deferred a full head-pair so the gpsimd broadcast (which
                    # can lag behind prelude DMA descriptor generation) is
                    # guaranteed done and never head-of-line-blocks the DVE FIFO
                    cur = []
                    for h2 in range(2):
                        asb = avsbp.tile([DK, I], F32, name=f"avsb{hp}_{h2}", tag="avsb")
                        nc.vector.tensor_copy(asb[:, :], av[h2][0:DK, :])
                        sums = nrmp.tile([1, I], F32, name=f"sm{hp}_{h2}", tag="sums", bufs=2)
                        nc.vector.tensor_copy(sums[:, :], av[h2][DK : DK + 1, :])
                        r = nrmp.tile([1, I], F32, name=f"rc{hp}_{h2}", tag="rc", bufs=2)
                        nc.vector.reciprocal_approx_fast(r[:, :], sums[:, :])
                        rb = nrmp.tile([DK, I], F32, name=f"rb{hp}_{h2}", tag="rb")
                        nc.gpsimd.partition_broadcast(rb[:, :], r[0:1, :])
                        cur.append((asb, rb))
                    for h2, (asb, rb) in enumerate(pending_mults):
                        nc.vector.tensor_mul(
                            attT[hp - 1][h2 * DK : (h2 + 1) * DK, :], asb[:, :], rb[:, :]
                        )
                    pending_mults = cur
                for h2, (asb, rb) in enumerate(pending_mults):
                    nc.vector.tensor_mul(
                        attT[NHP - 1][h2 * DK : (h2 + 1) * DK, :], asb[:, :], rb[:, :]
                    )

                # keep PE warm across the normalize -> projection handoff
                wps2 = smmp.tile([128, I], F32, name="wps2", tag="smm")
                for _ in range(10):
                    nc.tensor.matmul(
                        wps2[:, 0:512],
                        warm_sb[:, 0:128],
                        warm_sb[:, :],
                        start=True,
                        stop=True,
                        skip_group_check=True,
                    )

            # bias broadcast emitted late so it doesn't block the hp-loop
            # normalize broadcasts in the GpSimd FIFO
            nc.gpsimd.partition_broadcast(bias_bc[:, :], bias_sb[0:1, :])

            # ---- output projection ----
            with tc.tile_pool(name="projp", bufs=4, space="PSUM") as projp:
                for ic in range(I // 128):
                    # interleave the two ec psum banks so consecutive
                    # accumulating matmuls alternate banks and pipeline
                    pp = [
                        projp.tile([128, 512], F32, name=f"pp{ic}_{ec}", tag="pp")
                        for ec in range(2)
                    ]
                    for dc in range(NHP):
                        for ec in range(2):
                            nc.tensor.matmul(
                                pp[ec][:, :],
                                attT[dc][:, ic * 128 : (ic + 1) * 128],
                                wT(dc)[:, ec * 512 : (ec + 1) * 512],
                                start=(dc == 0),
                                stop=(dc == NHP - 1),
                                skip_group_check=True,
                            )
                    for ec in range(2):
                        fin = finp.tile([128, 512], F32, name=f"fin{ic}_{ec}", tag="fin")
                        nc.vector.tensor_add(
                            fin[:, :], pp[ec][:, :], bias_bc[:, ec * 512 : (ec + 1) * 512]
                        )
                        nc.sync.dma_start(
                            out=out[
                                ic * 128 : (ic + 1) * 128, ec * 512 : (ec + 1) * 512
                            ],
                            in_=fin[:, :],
                        )
    return nc


_NC_CACHE = {}


def _get_nc():
    if "nc" not in _NC_CACHE:
        nc = bacc.Bacc("TRN2", target_bir_lowering=False, debug=False)
        build(nc)
        nc.compile()
        _NC_CACHE["nc"] = nc
    return _NC_CACHE["nc"]


def kernel(q, k, v, W_out, b_out, _trace=False, _trace_kwargs=None):
    q = np.asarray(q, dtype=np.float32)
    k = np.asarray(k, dtype=np.float32)
    v = np.asarray(v, dtype=np.float32)
    W_out = np.ascontiguousarray(np.asarray(W_out, dtype=np.float32))
    b_out = np.ascontiguousarray(np.asarray(b_out, dtype=np.float32))

    nc = _get_nc()
    in_maps = []
    for c in range(8):
        bi, half = c // 2, c % 2
        in_maps.append(
            {
                "q": np.ascontiguousarray(q[bi, half * I : (half + 1) * I, :]),
                "k": np.ascontiguousarray(k[bi]),
                "v": np.ascontiguousarray(v[bi]),
                "w": W_out,
                "b": b_out,
            }
        )
    res = run_bass_kernel_spmd(
        nc,
        in_maps,
        core_ids=list(range(8)),
        trace=_trace,
        **(_trace_kwargs or {}),
    )
    out = np.empty((B, S, DM), np.float32)
    for c in range(8):
        bi, half = c // 2, c % 2
        out[bi, half * I : (half + 1) * I, :] = res.results[c]["out"]
    if _trace:
        return out, res
    return out
